# revision 11
# baseline (speedup 1.0000x reference)
"""Bass/Tile TRN2 kernel for nn_ExpressionAttentionLayer.

Math per batch b (B=8, G=2048, D=64):
    K_fused = concat([K_gene, K_expr], -1) @ WK_w.T + WK_b      # (G, D)
    Q_fused = concat([Q_gene, Q_expr], -1) @ WQ_w.T + WQ_b      # (G, D)
    A       = softmax(Q_fused @ K_fused.T / sqrt(D), axis=-1)
    out     = (A * M) @ V_expr                                   # (G, D)

Sharding: data-parallel over batch; core i handles batch i (B == n_cores == 8).
No collectives.

Per-core dataflow:
  - Build QcatT/KcatT [2D=128, G] via PE transposes of the natural [G, D] tiles
    (gene -> partitions 0:64, expr -> 64:128).
  - Project on PE: F_T[d, g] = W_T.T @ catT  -> K_fusedT/Q_fusedT [64, G] (+bias).
  - Per q-tile (128 rows): logits psum = Q_fusedT_tile.T @ K_fusedT (contract d),
    exp on ScalarE with fused row-sum (scale=1/sqrt(D) folded into the
    activation; |logits| <~ 7 so no max-subtraction is needed in fp32),
    multiply by the streamed M tile on VectorE, PE-transpose expM into
    [k, q] layout, and accumulate out^T = V.T-free matmuls over k-tiles.
  - Apply the softmax denominator as a per-partition reciprocal scale while
    copying the re-transposed output, then DMA out.
"""

from contextlib import ExitStack

import numpy as np

import concourse.bass as bass
import concourse.tile as tile
from concourse import bacc, mybir
from concourse.bass_utils import run_bass_kernel_spmd
from concourse.masks import make_identity

B, G, D = 8, 2048, 64
P = 128
NT = G // P  # 16 tiles of 128 rows
F32 = mybir.dt.float32
AF = mybir.ActivationFunctionType

N_CORES = 8


def _emit(ctx: ExitStack, tc: tile.TileContext, io: dict):
    nc = tc.nc

    singles = ctx.enter_context(tc.tile_pool(name="singles", bufs=1))
    ld = ctx.enter_context(tc.tile_pool(name="ld", bufs=4))

    # PSUM pools (8 banks total: 2*2 + 2 + 1 + 1 = 8)
    ps_l = ctx.enter_context(tc.tile_pool(name="ps_l", bufs=2, space="PSUM"))
    ps_t = ctx.enter_context(tc.tile_pool(name="ps_t", bufs=2, space="PSUM"))
    ps_o = ctx.enter_context(tc.tile_pool(name="ps_o", bufs=1, space="PSUM"))
    ps_f = ctx.enter_context(tc.tile_pool(name="ps_f", bufs=1, space="PSUM"))

    identity = singles.tile([P, P], F32)
    make_identity(nc, identity[:])

    # ---- weights: WK_w/WQ_w are [D, 2D]; we need transposed [k, d] lhsT
    # halves (gene k=0:64, expr k=64:128), both based at partition 0 so they
    # can pair with the partition-0 geneT/exprT tiles in accumulating matmuls.
    wk_gT = singles.tile([D, D], F32, tag="wk_gT")
    wk_eT = singles.tile([D, D], F32, tag="wk_eT")
    wq_gT = singles.tile([D, D], F32, tag="wq_gT")
    wq_eT = singles.tile([D, D], F32, tag="wq_eT")
    with nc.allow_non_contiguous_dma(reason="one-time 64KB weight transpose"):
        nc.sync.dma_start(wk_gT[:], io["WK_w"][:, 0:D].rearrange("d k -> k d"))
        nc.sync.dma_start(wk_eT[:], io["WK_w"][:, D : 2 * D].rearrange("d k -> k d"))
        nc.sync.dma_start(wq_gT[:], io["WQ_w"][:, 0:D].rearrange("d k -> k d"))
        nc.sync.dma_start(wq_eT[:], io["WQ_w"][:, D : 2 * D].rearrange("d k -> k d"))
    wkb = singles.tile([D, 1], F32, tag="wkb")
    wqb = singles.tile([D, 1], F32, tag="wqb")
    nc.sync.dma_start(wkb[:], io["WK_b"][:, None])
    nc.sync.dma_start(wqb[:], io["WQ_b"][:, None])

    # ---- V in [128, NT, D] (g on partitions, natural layout) ----
    v_sb = singles.tile([P, NT, D], F32, tag="v")
    nc.sync.dma_start(v_sb[:], io["V_expr"].rearrange("(t p) d -> p t d", p=P))

    # ---- transpose Q/K gene+expr into [D, G] (d on partitions) ----
    kgT = singles.tile([D, G], F32, tag="kgT")
    keT = singles.tile([D, G], F32, tag="keT")
    qgT = singles.tile([D, G], F32, tag="qgT")
    qeT = singles.tile([D, G], F32, tag="qeT")
    for src, dstT in (
        (io["K_gene"], kgT),
        (io["K_expr"], keT),
        (io["Q_gene"], qgT),
        (io["Q_expr"], qeT),
    ):
        for j in range(4):  # 4 batches of 4 g-tiles
            ps = ps_t.tile([P, 4 * P], F32, tag="ps_t", name="ps_tr")[:D]
            for i in range(4):
                t = 4 * j + i
                lt = ld.tile([P, D], F32, tag="ld_t")
                nc.sync.dma_start(lt[:], src[t * P : (t + 1) * P, :])
                nc.tensor.transpose(ps[:, i * P : (i + 1) * P], lt[:], identity[:])
            nc.vector.tensor_copy(dstT[:, j * 512 : (j + 1) * 512], ps[:])

    # ---- fused projections -> K_fusedT / Q_fusedT [D, G] ----
    # F_T = Wg.T-style contraction over the gene half + expr half, accumulated.
    kfT = singles.tile([D, G], F32, tag="kfT")
    qfT = singles.tile([D, G], F32, tag="qfT")
    for gT, eT, wgT, weT, b_sb, fT in (
        (kgT, keT, wk_gT, wk_eT, wkb, kfT),
        (qgT, qeT, wq_gT, wq_eT, wqb, qfT),
    ):
        for n in range(4):
            ps = ps_o.tile([D, 512], F32, tag="ps_o", name="ps_pj")
            nc.tensor.matmul(
                ps[:], wgT[:], gT[:, n * 512 : (n + 1) * 512], start=True, stop=False
            )
            nc.tensor.matmul(
                ps[:], weT[:], eT[:, n * 512 : (n + 1) * 512], start=False, stop=True
            )
            nc.scalar.activation(
                fT[:, n * 512 : (n + 1) * 512], ps[:], AF.Identity, bias=b_sb[:, 0:1]
            )

    # ---- main attention loop ----
    mpool = ctx.enter_context(tc.tile_pool(name="mpool", bufs=3))
    epool = ctx.enter_context(tc.tile_pool(name="epool", bufs=2))
    empool = ctx.enter_context(tc.tile_pool(name="empool", bufs=2))
    tpool = ctx.enter_context(tc.tile_pool(name="tpool", bufs=2))
    opool = ctx.enter_context(tc.tile_pool(name="opool", bufs=2))
    rpool = ctx.enter_context(tc.tile_pool(name="rpool", bufs=2))
    rspool = ctx.enter_context(tc.tile_pool(name="rspool", bufs=4))

    m_ap = io["M"]
    out_r = io["out"].rearrange("(t p) d -> p t d", p=P)
    scale = 1.0 / np.sqrt(np.float32(D))

    for grp in range(4):
        emt = tpool.tile([P, NT, 512], F32, tag="emt")  # expM^T for 4 q-tiles
        recips = rpool.tile([P, 4], F32, tag="recips")
        for i in range(4):
            qt = 4 * grp + i
            mt = mpool.tile([P, G], F32, tag="m")
            nc.sync.dma_start(mt[:], m_ap[qt * P : (qt + 1) * P, :])

            ex = epool.tile([P, G], F32, tag="ex")
            rs = [
                rspool.tile([P, 1], F32, tag=f"rs{h}", name=f"rs{h}") for h in range(2)
            ]
            # logits in two [128, 1024] psum tiles (2 banks each) so the next
            # q-tile's matmuls can start while this one's exp drains.
            for h in range(2):
                psl = ps_l.tile([P, 1024], F32, tag="ps_l")
                for n in range(2):
                    nc.tensor.matmul(
                        psl[:, n * 512 : (n + 1) * 512],
                        qfT[:, qt * P : (qt + 1) * P],
                        kfT[:, (2 * h + n) * 512 : (2 * h + n + 1) * 512],
                        start=True,
                        stop=True,
                    )
                nc.scalar.activation(
                    ex[:, h * 1024 : (h + 1) * 1024],
                    psl[:],
                    AF.Exp,
                    scale=float(scale),
                    accum_out=rs[h][:],
                )
            rsum = rspool.tile([P, 1], F32, tag="rsum")
            nc.vector.tensor_add(rsum[:], rs[0][:], rs[1][:])
            nc.vector.reciprocal(recips[:, i : i + 1], rsum[:])

            em = empool.tile([P, G], F32, tag="em")
            nc.vector.tensor_mul(em[:], ex[:], mt[:])

            for j in range(4):
                pst = ps_t.tile([P, 4 * P], F32, tag="ps_t")
                for k in range(4):
                    kt = 4 * j + k
                    nc.tensor.transpose(
                        pst[:, k * P : (k + 1) * P],
                        em[:, kt * P : (kt + 1) * P],
                        identity[:],
                    )
                # Pin the copy engine per k-group so each AV matmul's rhs
                # slice has a single writer engine (bounds its sync waits).
                eng = nc.vector if j < 2 else nc.scalar
                if eng is nc.vector:
                    eng.tensor_copy(
                        emt[:, 4 * j : 4 * j + 4, i * P : (i + 1) * P],
                        pst[:].rearrange("p (a b) -> p a b", a=4),
                    )
                else:
                    nc.scalar.copy(
                        emt[:, 4 * j : 4 * j + 4, i * P : (i + 1) * P],
                        pst[:].rearrange("p (a b) -> p a b", a=4),
                    )

        # out^T[d, q] for the 4 q-tiles of this group
        pso = ps_o.tile([D, 512], F32, tag="ps_o")
        for kt in range(NT):
            nc.tensor.matmul(
                pso[:],
                v_sb[:, kt, :],
                emt[:, kt, :],
                start=(kt == 0),
                stop=(kt == NT - 1),
            )
        otT = opool.tile([D, 512], F32, tag="otT")
        nc.vector.tensor_copy(otT[:], pso[:])

        psf = ps_f.tile([P, 4, D], F32, tag="ps_f")
        ob = opool.tile([P, 4, D], F32, tag="ob")
        for i in range(4):
            nc.tensor.transpose(
                psf[:, i, :], otT[:, i * P : (i + 1) * P], identity[:D, :D]
            )
            # apply softmax denominator while copying out of PSUM
            nc.scalar.activation(
                ob[:, i, :], psf[:, i, :], AF.Copy, bias=0.0, scale=recips[:, i : i + 1]
            )
        nc.sync.dma_start(out_r[:, 4 * grp : 4 * grp + 4, :], ob[:])


def _build():
    # Bacc (not plain Bass): its compile() legalizes sync waits
    # (move_matmul_waits_to_ldweights + generate_event_semaphores) which
    # walrus codegen requires (max 1 wait per instruction).
    nc = bacc.Bacc("TRN2", target_bir_lowering=False, debug=False)
    io = {}
    for name in ("Q_gene", "K_gene", "Q_expr", "K_expr", "V_expr"):
        io[name] = nc.dram_tensor(name, [G, D], F32, kind="ExternalInput").ap()
    io["M"] = nc.dram_tensor("M", [G, G], F32, kind="ExternalInput").ap()
    for name in ("WK_w", "WQ_w"):
        io[name] = nc.dram_tensor(name, [D, 2 * D], F32, kind="ExternalInput").ap()
    for name in ("WK_b", "WQ_b"):
        io[name] = nc.dram_tensor(name, [D], F32, kind="ExternalInput").ap()
    io["out"] = nc.dram_tensor("out", [G, D], F32, kind="ExternalOutput").ap()

    with tile.TileContext(nc) as tc:
        with ExitStack() as ctx:
            _emit(ctx, tc, io)
    nc.compile()
    return nc


_NC = None


def _get_nc():
    global _NC
    if _NC is None:
        _NC = _build()
    return _NC


def kernel(**inputs) -> np.ndarray:
    return run_kernel_with_results(**inputs)[0]


def run_kernel_with_results(trace=False, **inputs):
    """Returns (full_output, BassKernelResults)."""
    nc = _get_nc()
    per_core_names = ("Q_gene", "K_gene", "Q_expr", "K_expr", "V_expr", "M")
    shared_names = ("WK_w", "WK_b", "WQ_w", "WQ_b")
    arrs = {k: np.ascontiguousarray(np.asarray(v), dtype=np.float32) for k, v in inputs.items()}
    in_maps = []
    for c in range(N_CORES):
        im = {n: arrs[n][c] for n in per_core_names}
        for n in shared_names:
            im[n] = arrs[n]
        in_maps.append(im)
    res = run_bass_kernel_spmd(nc, in_maps, list(range(N_CORES)), trace=trace)
    out = np.stack([res.results[c]["out"] for c in range(N_CORES)], axis=0)
    return out.astype(np.float32), res


# revision 14
# speedup vs baseline: 1.0350x; 1.0350x over previous
"""Bass/Tile TRN2 kernel for nn_ExpressionAttentionLayer.

Math per batch b (B=8, G=2048, D=64):
    K_fused = concat([K_gene, K_expr], -1) @ WK_w.T + WK_b      # (G, D)
    Q_fused = concat([Q_gene, Q_expr], -1) @ WQ_w.T + WQ_b      # (G, D)
    A       = softmax(Q_fused @ K_fused.T / sqrt(D), axis=-1)
    out     = (A * M) @ V_expr                                   # (G, D)

Sharding: data-parallel over batch; core i handles batch i (B == n_cores == 8).
No collectives.

Per-core dataflow:
  - Build QcatT/KcatT [2D=128, G] via PE transposes of the natural [G, D] tiles
    (gene -> partitions 0:64, expr -> 64:128).
  - Project on PE: F_T[d, g] = W_T.T @ catT  -> K_fusedT/Q_fusedT [64, G] (+bias).
  - Per q-tile (128 rows): logits psum = Q_fusedT_tile.T @ K_fusedT (contract d),
    exp on ScalarE with fused row-sum (scale=1/sqrt(D) folded into the
    activation; |logits| <~ 7 so no max-subtraction is needed in fp32),
    multiply by the streamed M tile on VectorE, PE-transpose expM into
    [k, q] layout, and accumulate out^T = V.T-free matmuls over k-tiles.
  - Apply the softmax denominator as a per-partition reciprocal scale while
    copying the re-transposed output, then DMA out.
"""

from contextlib import ExitStack

import numpy as np

import concourse.bass as bass
import concourse.tile as tile
from concourse import bacc, mybir
from concourse.bass_utils import run_bass_kernel_spmd
from concourse.masks import make_identity

B, G, D = 8, 2048, 64
P = 128
NT = G // P  # 16 tiles of 128 rows
F32 = mybir.dt.float32
AF = mybir.ActivationFunctionType

N_CORES = 8


def _emit(ctx: ExitStack, tc: tile.TileContext, io: dict):
    nc = tc.nc

    singles = ctx.enter_context(tc.tile_pool(name="singles", bufs=1))
    ld = ctx.enter_context(tc.tile_pool(name="ld", bufs=4))

    # PSUM pools (8 banks total: 2*2 + 2 + 1 + 1 = 8)
    ps_l = ctx.enter_context(tc.tile_pool(name="ps_l", bufs=2, space="PSUM"))
    ps_t = ctx.enter_context(tc.tile_pool(name="ps_t", bufs=2, space="PSUM"))
    ps_o = ctx.enter_context(tc.tile_pool(name="ps_o", bufs=1, space="PSUM"))
    ps_f = ctx.enter_context(tc.tile_pool(name="ps_f", bufs=1, space="PSUM"))

    identity = singles.tile([P, P], F32)
    make_identity(nc, identity[:])

    # ---- weights: WK_w/WQ_w are [D, 2D]; we need transposed [k, d] lhsT
    # halves (gene k=0:64, expr k=64:128), both based at partition 0 so they
    # can pair with the partition-0 geneT/exprT tiles in accumulating matmuls.
    wk_gT = singles.tile([D, D], F32, tag="wk_gT")
    wk_eT = singles.tile([D, D], F32, tag="wk_eT")
    wq_gT = singles.tile([D, D], F32, tag="wq_gT")
    wq_eT = singles.tile([D, D], F32, tag="wq_eT")
    with nc.allow_non_contiguous_dma(reason="one-time 64KB weight transpose"):
        nc.sync.dma_start(wk_gT[:], io["WK_w"][:, 0:D].rearrange("d k -> k d"))
        nc.sync.dma_start(wk_eT[:], io["WK_w"][:, D : 2 * D].rearrange("d k -> k d"))
        nc.sync.dma_start(wq_gT[:], io["WQ_w"][:, 0:D].rearrange("d k -> k d"))
        nc.sync.dma_start(wq_eT[:], io["WQ_w"][:, D : 2 * D].rearrange("d k -> k d"))
    wkb = singles.tile([D, 1], F32, tag="wkb")
    wqb = singles.tile([D, 1], F32, tag="wqb")
    nc.sync.dma_start(wkb[:], io["WK_b"][:, None])
    nc.sync.dma_start(wqb[:], io["WQ_b"][:, None])

    # ---- V in [128, NT, D] (g on partitions, natural layout) ----
    v_sb = singles.tile([P, NT, D], F32, tag="v")
    nc.sync.dma_start(v_sb[:], io["V_expr"].rearrange("(t p) d -> p t d", p=P))

    # ---- transpose Q/K gene+expr into [D, G] (d on partitions) ----
    # One big DMA per tensor (batching kills per-dma SP descriptor cost),
    # then transposes as REGULAR matmuls vs identity (lhsT.T @ I) — unlike
    # transpose-mode these count as PE activity, so HAM un-throttles the
    # PE clock to 2.4 GHz.
    kgT = singles.tile([D, G], F32, tag="kgT")
    keT = singles.tile([D, G], F32, tag="keT")
    qgT = singles.tile([D, G], F32, tag="qgT")
    qeT = singles.tile([D, G], F32, tag="qeT")
    for src, dstT in (
        (io["K_gene"], kgT),
        (io["K_expr"], keT),
        (io["Q_gene"], qgT),
        (io["Q_expr"], qeT),
    ):
        big = ld.tile([P, NT, D], F32, tag="ld_t")
        nc.sync.dma_start(big[:], src.rearrange("(t p) d -> p t d", p=P))
        for j in range(4):  # 4 batches of 4 g-tiles
            ps = ps_t.tile([P, 4 * P], F32, tag="ps_t", name="ps_tr")[:D]
            for i in range(4):
                t = 4 * j + i
                nc.tensor.matmul(
                    ps[:, i * P : (i + 1) * P],
                    big[:, t, :],
                    identity[:],
                    start=True,
                    stop=True,
                )
            nc.vector.tensor_copy(dstT[:, j * 512 : (j + 1) * 512], ps[:])

    # ---- fused projections -> K_fusedT / Q_fusedT [D, G] ----
    # F_T = Wg.T-style contraction over the gene half + expr half, accumulated.
    kfT = singles.tile([D, G], F32, tag="kfT")
    qfT = singles.tile([D, G], F32, tag="qfT")
    for gT, eT, wgT, weT, b_sb, fT in (
        (kgT, keT, wk_gT, wk_eT, wkb, kfT),
        (qgT, qeT, wq_gT, wq_eT, wqb, qfT),
    ):
        for n in range(4):
            ps = ps_o.tile([D, 512], F32, tag="ps_o", name="ps_pj")
            nc.tensor.matmul(
                ps[:], wgT[:], gT[:, n * 512 : (n + 1) * 512], start=True, stop=False
            )
            nc.tensor.matmul(
                ps[:], weT[:], eT[:, n * 512 : (n + 1) * 512], start=False, stop=True
            )
            nc.scalar.activation(
                fT[:, n * 512 : (n + 1) * 512], ps[:], AF.Identity, bias=b_sb[:, 0:1]
            )

    # ---- main attention loop ----
    mpool = ctx.enter_context(tc.tile_pool(name="mpool", bufs=3))
    epool = ctx.enter_context(tc.tile_pool(name="epool", bufs=2))
    empool = ctx.enter_context(tc.tile_pool(name="empool", bufs=2))
    tpool = ctx.enter_context(tc.tile_pool(name="tpool", bufs=2))
    opool = ctx.enter_context(tc.tile_pool(name="opool", bufs=2))
    rpool = ctx.enter_context(tc.tile_pool(name="rpool", bufs=2))
    rspool = ctx.enter_context(tc.tile_pool(name="rspool", bufs=4))

    m_ap = io["M"]
    out_r = io["out"].rearrange("(t p) d -> p t d", p=P)
    scale = 1.0 / np.sqrt(np.float32(D))

    for grp in range(4):
        emt = tpool.tile([P, NT, 512], F32, tag="emt")  # expM^T for 4 q-tiles
        recips = rpool.tile([P, 4], F32, tag="recips")
        for i in range(4):
            qt = 4 * grp + i
            mt = mpool.tile([P, G], F32, tag="m")
            nc.sync.dma_start(mt[:], m_ap[qt * P : (qt + 1) * P, :])

            ex = epool.tile([P, G], F32, tag="ex")
            rs = [
                rspool.tile([P, 1], F32, tag=f"rs{h}", name=f"rs{h}") for h in range(2)
            ]
            # logits in two [128, 1024] psum tiles (2 banks each) so the next
            # q-tile's matmuls can start while this one's exp drains.
            for h in range(2):
                psl = ps_l.tile([P, 1024], F32, tag="ps_l")
                for n in range(2):
                    nc.tensor.matmul(
                        psl[:, n * 512 : (n + 1) * 512],
                        qfT[:, qt * P : (qt + 1) * P],
                        kfT[:, (2 * h + n) * 512 : (2 * h + n + 1) * 512],
                        start=True,
                        stop=True,
                    )
                nc.scalar.activation(
                    ex[:, h * 1024 : (h + 1) * 1024],
                    psl[:],
                    AF.Exp,
                    scale=float(scale),
                    accum_out=rs[h][:],
                )
            rsum = rspool.tile([P, 1], F32, tag="rsum")
            nc.vector.tensor_add(rsum[:], rs[0][:], rs[1][:])
            nc.vector.reciprocal(recips[:, i : i + 1], rsum[:])

            em = empool.tile([P, G], F32, tag="em")
            nc.vector.tensor_mul(em[:], ex[:], mt[:])

            for j in range(4):
                pst = ps_t.tile([P, 4 * P], F32, tag="ps_t")
                for k in range(4):
                    kt = 4 * j + k
                    nc.tensor.matmul(
                        pst[:, k * P : (k + 1) * P],
                        em[:, kt * P : (kt + 1) * P],
                        identity[:],
                        start=True,
                        stop=True,
                    )
                # Pin the copy engine per k-group so each AV matmul's rhs
                # slice has a single writer engine (bounds its sync waits).
                eng = nc.vector if j < 2 else nc.scalar
                if eng is nc.vector:
                    eng.tensor_copy(
                        emt[:, 4 * j : 4 * j + 4, i * P : (i + 1) * P],
                        pst[:].rearrange("p (a b) -> p a b", a=4),
                    )
                else:
                    nc.scalar.copy(
                        emt[:, 4 * j : 4 * j + 4, i * P : (i + 1) * P],
                        pst[:].rearrange("p (a b) -> p a b", a=4),
                    )

        # out^T[d, q] for the 4 q-tiles of this group
        pso = ps_o.tile([D, 512], F32, tag="ps_o")
        for kt in range(NT):
            nc.tensor.matmul(
                pso[:],
                v_sb[:, kt, :],
                emt[:, kt, :],
                start=(kt == 0),
                stop=(kt == NT - 1),
            )
        otT = opool.tile([D, 512], F32, tag="otT")
        nc.vector.tensor_copy(otT[:], pso[:])

        psf = ps_f.tile([P, 4, D], F32, tag="ps_f")
        ob = opool.tile([P, 4, D], F32, tag="ob")
        for i in range(4):
            nc.tensor.matmul(
                psf[:, i, :],
                otT[:, i * P : (i + 1) * P],
                identity[:D, :D],
                start=True,
                stop=True,
            )
            # apply softmax denominator while copying out of PSUM
            nc.scalar.activation(
                ob[:, i, :], psf[:, i, :], AF.Copy, bias=0.0, scale=recips[:, i : i + 1]
            )
        nc.sync.dma_start(out_r[:, 4 * grp : 4 * grp + 4, :], ob[:])


def _build():
    # Bacc (not plain Bass): its compile() legalizes sync waits
    # (move_matmul_waits_to_ldweights + generate_event_semaphores) which
    # walrus codegen requires (max 1 wait per instruction).
    nc = bacc.Bacc("TRN2", target_bir_lowering=False, debug=False)
    io = {}
    for name in ("Q_gene", "K_gene", "Q_expr", "K_expr", "V_expr"):
        io[name] = nc.dram_tensor(name, [G, D], F32, kind="ExternalInput").ap()
    io["M"] = nc.dram_tensor("M", [G, G], F32, kind="ExternalInput").ap()
    for name in ("WK_w", "WQ_w"):
        io[name] = nc.dram_tensor(name, [D, 2 * D], F32, kind="ExternalInput").ap()
    for name in ("WK_b", "WQ_b"):
        io[name] = nc.dram_tensor(name, [D], F32, kind="ExternalInput").ap()
    io["out"] = nc.dram_tensor("out", [G, D], F32, kind="ExternalOutput").ap()

    with tile.TileContext(nc) as tc:
        with ExitStack() as ctx:
            _emit(ctx, tc, io)
    nc.compile()
    return nc


_NC = None


def _get_nc():
    global _NC
    if _NC is None:
        _NC = _build()
    return _NC


def kernel(**inputs) -> np.ndarray:
    return run_kernel_with_results(**inputs)[0]


def run_kernel_with_results(trace=False, **inputs):
    """Returns (full_output, BassKernelResults)."""
    nc = _get_nc()
    per_core_names = ("Q_gene", "K_gene", "Q_expr", "K_expr", "V_expr", "M")
    shared_names = ("WK_w", "WK_b", "WQ_w", "WQ_b")
    arrs = {k: np.ascontiguousarray(np.asarray(v), dtype=np.float32) for k, v in inputs.items()}
    in_maps = []
    for c in range(N_CORES):
        im = {n: arrs[n][c] for n in per_core_names}
        for n in shared_names:
            im[n] = arrs[n]
        in_maps.append(im)
    res = run_bass_kernel_spmd(nc, in_maps, list(range(N_CORES)), trace=trace)
    out = np.stack([res.results[c]["out"] for c in range(N_CORES)], axis=0)
    return out.astype(np.float32), res


# revision 15
# speedup vs baseline: 1.9175x; 1.8527x over previous
"""Bass/Tile TRN2 kernel for nn_ExpressionAttentionLayer.

Math per batch b (B=8, G=2048, D=64):
    K_fused = concat([K_gene, K_expr], -1) @ WK_w.T + WK_b      # (G, D)
    Q_fused = concat([Q_gene, Q_expr], -1) @ WQ_w.T + WQ_b      # (G, D)
    A       = softmax(Q_fused @ K_fused.T / sqrt(D), axis=-1)
    out     = (A * M) @ V_expr                                   # (G, D)

Sharding: data-parallel over batch; core i handles batch i (B == n_cores == 8).
No collectives.

Per-core dataflow:
  - Transpose Q/K gene+expr into [D, G] via PE transpose-mode (1-pass for
    fp32 on cayman); project on PE in bf16 -> K_fusedT/Q_fusedT [64, G] bf16.
  - Per q-tile (128 rows): logits psum(fp32) = Q_tile.T @ K_fusedT (bf16,
    contract d=64), exp on ScalarE with fused row-sum (scale=1/sqrt(D)
    folded in; |logits| <~ 7 so no max-subtraction needed), multiply by the
    streamed M tile on VectorE (bf16 out), PE-transpose expM (bf16 psum),
    copy to [k, q] sbuf tiles, accumulate out^T over k-tiles on PE (bf16).
  - Apply the softmax denominator as a per-partition reciprocal scale while
    copying the re-transposed fp32 output, then DMA out.

fp32 matmuls cost 2 PE passes on trn2; every matmul here runs bf16 inputs
with fp32 PSUM accumulation except nothing — accuracy comes from fp32
softmax statistics and fp32 accumulation.
"""

from contextlib import ExitStack

import numpy as np

import concourse.bass as bass
import concourse.tile as tile
from concourse import bacc, mybir
from concourse.bass_utils import run_bass_kernel_spmd
from concourse.masks import make_identity

B, G, D = 8, 2048, 64
P = 128
NT = G // P  # 16 tiles of 128 rows
F32 = mybir.dt.float32
BF16 = mybir.dt.bfloat16
AF = mybir.ActivationFunctionType

N_CORES = 8


def _emit(ctx: ExitStack, tc: tile.TileContext, io: dict):
    nc = tc.nc

    singles = ctx.enter_context(tc.tile_pool(name="singles", bufs=1))
    ld = ctx.enter_context(tc.tile_pool(name="ld", bufs=4))

    # PSUM pools (8 banks total: 2*2 + 2 + 1 + 1 = 8)
    ps_l = ctx.enter_context(tc.tile_pool(name="ps_l", bufs=2, space="PSUM"))
    ps_t = ctx.enter_context(tc.tile_pool(name="ps_t", bufs=2, space="PSUM"))
    ps_o = ctx.enter_context(tc.tile_pool(name="ps_o", bufs=1, space="PSUM"))
    ps_f = ctx.enter_context(tc.tile_pool(name="ps_f", bufs=1, space="PSUM"))

    identity = singles.tile([P, P], F32)
    make_identity(nc, identity[:])
    identity_bf = singles.tile([P, P], BF16)
    nc.vector.tensor_copy(identity_bf[:], identity[:])

    # ---- weights: WK_w/WQ_w are [D, 2D]; transposed [k, d] halves in bf16 ----
    wk_gT = singles.tile([D, D], F32, tag="wk_gT")
    wk_eT = singles.tile([D, D], F32, tag="wk_eT")
    wq_gT = singles.tile([D, D], F32, tag="wq_gT")
    wq_eT = singles.tile([D, D], F32, tag="wq_eT")
    with nc.allow_non_contiguous_dma(reason="one-time 64KB weight transpose"):
        nc.sync.dma_start(wk_gT[:], io["WK_w"][:, 0:D].rearrange("d k -> k d"))
        nc.sync.dma_start(wk_eT[:], io["WK_w"][:, D : 2 * D].rearrange("d k -> k d"))
        nc.sync.dma_start(wq_gT[:], io["WQ_w"][:, 0:D].rearrange("d k -> k d"))
        nc.sync.dma_start(wq_eT[:], io["WQ_w"][:, D : 2 * D].rearrange("d k -> k d"))
    wk_gTb = singles.tile([D, D], BF16, tag="wk_gTb")
    wk_eTb = singles.tile([D, D], BF16, tag="wk_eTb")
    wq_gTb = singles.tile([D, D], BF16, tag="wq_gTb")
    wq_eTb = singles.tile([D, D], BF16, tag="wq_eTb")
    nc.vector.tensor_copy(wk_gTb[:], wk_gT[:])
    nc.vector.tensor_copy(wk_eTb[:], wk_eT[:])
    nc.vector.tensor_copy(wq_gTb[:], wq_gT[:])
    nc.vector.tensor_copy(wq_eTb[:], wq_eT[:])
    wkb = singles.tile([D, 1], F32, tag="wkb")
    wqb = singles.tile([D, 1], F32, tag="wqb")
    nc.sync.dma_start(wkb[:], io["WK_b"][:, None])
    nc.sync.dma_start(wqb[:], io["WQ_b"][:, None])

    # ---- V in [128, NT, D] (g on partitions), cast to bf16 for the AV matmul
    v_sb = singles.tile([P, NT, D], F32, tag="v")
    nc.sync.dma_start(v_sb[:], io["V_expr"].rearrange("(t p) d -> p t d", p=P))
    v_bf = singles.tile([P, NT, D], BF16, tag="v_bf")
    nc.vector.tensor_copy(v_bf[:], v_sb[:])

    # ---- transpose Q/K gene+expr into bf16 [D, G] (d on partitions) ----
    kgT = singles.tile([D, G], BF16, tag="kgT")
    keT = singles.tile([D, G], BF16, tag="keT")
    qgT = singles.tile([D, G], BF16, tag="qgT")
    qeT = singles.tile([D, G], BF16, tag="qeT")
    for src, dstT in (
        (io["K_gene"], kgT),
        (io["K_expr"], keT),
        (io["Q_gene"], qgT),
        (io["Q_expr"], qeT),
    ):
        big = ld.tile([P, NT, D], F32, tag="ld_t")
        nc.sync.dma_start(big[:], src.rearrange("(t p) d -> p t d", p=P))
        for j in range(4):  # 4 batches of 4 g-tiles
            ps = ps_t.tile([P, 4 * P], F32, tag="ps_t", name="ps_tr")[:D]
            for i in range(4):
                t = 4 * j + i
                nc.tensor.transpose(ps[:, i * P : (i + 1) * P], big[:, t, :], identity[:])
            nc.vector.tensor_copy(dstT[:, j * 512 : (j + 1) * 512], ps[:])

    # ---- fused projections -> K_fusedT / Q_fusedT [D, G] bf16 ----
    kfT = singles.tile([D, G], BF16, tag="kfT")
    qfT = singles.tile([D, G], BF16, tag="qfT")
    for gT, eT, wgT, weT, b_sb, fT in (
        (kgT, keT, wk_gTb, wk_eTb, wkb, kfT),
        (qgT, qeT, wq_gTb, wq_eTb, wqb, qfT),
    ):
        for n in range(4):
            ps = ps_o.tile([D, 512], F32, tag="ps_o", name="ps_pj")
            nc.tensor.matmul(
                ps[:], wgT[:], gT[:, n * 512 : (n + 1) * 512], start=True, stop=False
            )
            nc.tensor.matmul(
                ps[:], weT[:], eT[:, n * 512 : (n + 1) * 512], start=False, stop=True
            )
            nc.scalar.activation(
                fT[:, n * 512 : (n + 1) * 512], ps[:], AF.Identity, bias=b_sb[:, 0:1]
            )

    # ---- main attention loop ----
    mpool = ctx.enter_context(tc.tile_pool(name="mpool", bufs=3))
    epool = ctx.enter_context(tc.tile_pool(name="epool", bufs=2))
    empool = ctx.enter_context(tc.tile_pool(name="empool", bufs=2))
    tpool = ctx.enter_context(tc.tile_pool(name="tpool", bufs=2))
    opool = ctx.enter_context(tc.tile_pool(name="opool", bufs=2))
    rpool = ctx.enter_context(tc.tile_pool(name="rpool", bufs=2))
    rspool = ctx.enter_context(tc.tile_pool(name="rspool", bufs=4))

    m_ap = io["M"]
    out_r = io["out"].rearrange("(t p) d -> p t d", p=P)
    scale = 1.0 / np.sqrt(np.float32(D))

    for grp in range(4):
        emt = tpool.tile([P, NT, 512], BF16, tag="emt")  # expM^T for 4 q-tiles
        recips = rpool.tile([P, 4], F32, tag="recips")
        for i in range(4):
            qt = 4 * grp + i
            mt = mpool.tile([P, G], F32, tag="m")
            nc.sync.dma_start(mt[:], m_ap[qt * P : (qt + 1) * P, :])

            ex = epool.tile([P, G], F32, tag="ex")
            rs = [
                rspool.tile([P, 1], F32, tag=f"rs{h}", name=f"rs{h}") for h in range(2)
            ]
            # logits in two [128, 1024] psum tiles (2 banks each) so the next
            # q-tile's matmuls can start while this one's exp drains.
            for h in range(2):
                psl = ps_l.tile([P, 1024], F32, tag="ps_l")
                for n in range(2):
                    nc.tensor.matmul(
                        psl[:, n * 512 : (n + 1) * 512],
                        qfT[:, qt * P : (qt + 1) * P],
                        kfT[:, (2 * h + n) * 512 : (2 * h + n + 1) * 512],
                        start=True,
                        stop=True,
                    )
                nc.scalar.activation(
                    ex[:, h * 1024 : (h + 1) * 1024],
                    psl[:],
                    AF.Exp,
                    scale=float(scale),
                    accum_out=rs[h][:],
                )
            rsum = rspool.tile([P, 1], F32, tag="rsum")
            nc.vector.tensor_add(rsum[:], rs[0][:], rs[1][:])
            nc.vector.reciprocal(recips[:, i : i + 1], rsum[:])

            em = empool.tile([P, G], BF16, tag="em")
            nc.vector.tensor_mul(em[:], ex[:], mt[:])

            for j in range(4):
                pst = ps_t.tile([P, 4 * P], BF16, tag="ps_t")
                for k in range(4):
                    kt = 4 * j + k
                    nc.tensor.transpose(
                        pst[:, k * P : (k + 1) * P],
                        em[:, kt * P : (kt + 1) * P],
                        identity_bf[:],
                    )
                # Pin the copy engine per k-group so each AV matmul's rhs
                # slice has a single writer engine (bounds its sync waits).
                if j < 2:
                    nc.vector.tensor_copy(
                        emt[:, 4 * j : 4 * j + 4, i * P : (i + 1) * P],
                        pst[:].rearrange("p (a b) -> p a b", a=4),
                    )
                else:
                    nc.scalar.copy(
                        emt[:, 4 * j : 4 * j + 4, i * P : (i + 1) * P],
                        pst[:].rearrange("p (a b) -> p a b", a=4),
                    )

        # out^T[d, q] for the 4 q-tiles of this group
        pso = ps_o.tile([D, 512], F32, tag="ps_o")
        for kt in range(NT):
            nc.tensor.matmul(
                pso[:],
                v_bf[:, kt, :],
                emt[:, kt, :],
                start=(kt == 0),
                stop=(kt == NT - 1),
            )
        otT = opool.tile([D, 512], F32, tag="otT")
        nc.vector.tensor_copy(otT[:], pso[:])

        psf = ps_f.tile([P, 4, D], F32, tag="ps_f")
        ob = opool.tile([P, 4, D], F32, tag="ob")
        for i in range(4):
            nc.tensor.transpose(
                psf[:, i, :], otT[:, i * P : (i + 1) * P], identity[:D, :D]
            )
            # apply softmax denominator while copying out of PSUM
            nc.scalar.activation(
                ob[:, i, :], psf[:, i, :], AF.Copy, bias=0.0, scale=recips[:, i : i + 1]
            )
        nc.sync.dma_start(out_r[:, 4 * grp : 4 * grp + 4, :], ob[:])


def _build():
    # Bacc (not plain Bass): its compile() legalizes sync waits
    # (move_matmul_waits_to_ldweights + generate_event_semaphores) which
    # walrus codegen requires (max 1 wait per instruction).
    nc = bacc.Bacc("TRN2", target_bir_lowering=False, debug=False)
    io = {}
    for name in ("Q_gene", "K_gene", "Q_expr", "K_expr", "V_expr"):
        io[name] = nc.dram_tensor(name, [G, D], F32, kind="ExternalInput").ap()
    io["M"] = nc.dram_tensor("M", [G, G], F32, kind="ExternalInput").ap()
    for name in ("WK_w", "WQ_w"):
        io[name] = nc.dram_tensor(name, [D, 2 * D], F32, kind="ExternalInput").ap()
    for name in ("WK_b", "WQ_b"):
        io[name] = nc.dram_tensor(name, [D], F32, kind="ExternalInput").ap()
    io["out"] = nc.dram_tensor("out", [G, D], F32, kind="ExternalOutput").ap()

    with tile.TileContext(nc) as tc:
        with ExitStack() as ctx:
            _emit(ctx, tc, io)
    nc.compile()
    return nc


_NC = None


def _get_nc():
    global _NC
    if _NC is None:
        _NC = _build()
    return _NC


def kernel(**inputs) -> np.ndarray:
    return run_kernel_with_results(**inputs)[0]


def run_kernel_with_results(trace=False, **inputs):
    """Returns (full_output, BassKernelResults)."""
    nc = _get_nc()
    per_core_names = ("Q_gene", "K_gene", "Q_expr", "K_expr", "V_expr", "M")
    shared_names = ("WK_w", "WK_b", "WQ_w", "WQ_b")
    arrs = {k: np.ascontiguousarray(np.asarray(v), dtype=np.float32) for k, v in inputs.items()}
    in_maps = []
    for c in range(N_CORES):
        im = {n: arrs[n][c] for n in per_core_names}
        for n in shared_names:
            im[n] = arrs[n]
        in_maps.append(im)
    res = run_bass_kernel_spmd(nc, in_maps, list(range(N_CORES)), trace=trace)
    out = np.stack([res.results[c]["out"] for c in range(N_CORES)], axis=0)
    return out.astype(np.float32), res


# revision 17
# speedup vs baseline: 2.4436x; 1.2744x over previous
"""Bass/Tile TRN2 kernel for nn_ExpressionAttentionLayer.

Math per batch b (B=8, G=2048, D=64):
    K_fused = concat([K_gene, K_expr], -1) @ WK_w.T + WK_b      # (G, D)
    Q_fused = concat([Q_gene, Q_expr], -1) @ WQ_w.T + WQ_b      # (G, D)
    A       = softmax(Q_fused @ K_fused.T / sqrt(D), axis=-1)
    out     = (A * M) @ V_expr                                   # (G, D)

Sharding: data-parallel over batch; core i handles batch i (B == n_cores == 8).
No collectives.

Per-core dataflow:
  - Transpose Q/K gene+expr into [D, G] via PE transpose-mode (1-pass for
    fp32 on cayman); project on PE in bf16 -> K_fusedT/Q_fusedT [64, G] bf16.
  - Per q-tile (128 rows): logits psum(fp32) = Q_tile.T @ K_fusedT (bf16,
    contract d=64), exp on ScalarE with fused row-sum (scale=1/sqrt(D)
    folded in; |logits| <~ 7 so no max-subtraction needed), multiply by the
    streamed M tile on VectorE (bf16 out), PE-transpose expM (bf16 psum),
    copy to [k, q] sbuf tiles, accumulate out^T over k-tiles on PE (bf16).
  - Apply the softmax denominator as a per-partition reciprocal scale while
    copying the re-transposed fp32 output, then DMA out.

fp32 matmuls cost 2 PE passes on trn2; every matmul here runs bf16 inputs
with fp32 PSUM accumulation except nothing — accuracy comes from fp32
softmax statistics and fp32 accumulation.
"""

from contextlib import ExitStack

import numpy as np

import concourse.bass as bass
import concourse.tile as tile
from concourse import bacc, mybir
from concourse.bass_utils import run_bass_kernel_spmd
from concourse.masks import make_identity

B, G, D = 8, 2048, 64
P = 128
NT = G // P  # 16 tiles of 128 rows
F32 = mybir.dt.float32
BF16 = mybir.dt.bfloat16
AF = mybir.ActivationFunctionType

N_CORES = 8


def _emit(ctx: ExitStack, tc: tile.TileContext, io: dict):
    nc = tc.nc

    singles = ctx.enter_context(tc.tile_pool(name="singles", bufs=1))
    ld = ctx.enter_context(tc.tile_pool(name="ld", bufs=4))

    # PSUM pools (8 banks total: 2*2 + 2 + 2 = 8)
    ps_l = ctx.enter_context(tc.tile_pool(name="ps_l", bufs=2, space="PSUM"))
    ps_t = ctx.enter_context(tc.tile_pool(name="ps_t", bufs=2, space="PSUM"))
    ps_o = ctx.enter_context(tc.tile_pool(name="ps_o", bufs=2, space="PSUM"))

    identity = singles.tile([P, P], F32)
    make_identity(nc, identity[:])
    identity_bf = singles.tile([P, P], BF16)
    nc.vector.tensor_copy(identity_bf[:], identity[:])

    # ---- weights: WK_w/WQ_w are [D, 2D]; transposed [k, d] halves in bf16 ----
    wk_gT = singles.tile([D, D], F32, tag="wk_gT")
    wk_eT = singles.tile([D, D], F32, tag="wk_eT")
    wq_gT = singles.tile([D, D], F32, tag="wq_gT")
    wq_eT = singles.tile([D, D], F32, tag="wq_eT")
    with nc.allow_non_contiguous_dma(reason="one-time 64KB weight transpose"):
        nc.sync.dma_start(wk_gT[:], io["WK_w"][:, 0:D].rearrange("d k -> k d"))
        nc.sync.dma_start(wk_eT[:], io["WK_w"][:, D : 2 * D].rearrange("d k -> k d"))
        nc.sync.dma_start(wq_gT[:], io["WQ_w"][:, 0:D].rearrange("d k -> k d"))
        nc.sync.dma_start(wq_eT[:], io["WQ_w"][:, D : 2 * D].rearrange("d k -> k d"))
    wk_gTb = singles.tile([D, D], BF16, tag="wk_gTb")
    wk_eTb = singles.tile([D, D], BF16, tag="wk_eTb")
    wq_gTb = singles.tile([D, D], BF16, tag="wq_gTb")
    wq_eTb = singles.tile([D, D], BF16, tag="wq_eTb")
    nc.vector.tensor_copy(wk_gTb[:], wk_gT[:])
    nc.vector.tensor_copy(wk_eTb[:], wk_eT[:])
    nc.vector.tensor_copy(wq_gTb[:], wq_gT[:])
    nc.vector.tensor_copy(wq_eTb[:], wq_eT[:])
    wkb = singles.tile([D, 1], F32, tag="wkb")
    wqb = singles.tile([D, 1], F32, tag="wqb")
    nc.sync.dma_start(wkb[:], io["WK_b"][:, None])
    nc.sync.dma_start(wqb[:], io["WQ_b"][:, None])

    # ---- V in [128, NT, D] (g on partitions), cast to bf16 for the AV matmul
    v_sb = singles.tile([P, NT, D], F32, tag="v")
    nc.sync.dma_start(v_sb[:], io["V_expr"].rearrange("(t p) d -> p t d", p=P))
    v_bf = singles.tile([P, NT, D], BF16, tag="v_bf")
    nc.vector.tensor_copy(v_bf[:], v_sb[:])

    # ---- transpose Q/K gene+expr into bf16 [D, G] (d on partitions) ----
    kgT = singles.tile([D, G], BF16, tag="kgT")
    keT = singles.tile([D, G], BF16, tag="keT")
    qgT = singles.tile([D, G], BF16, tag="qgT")
    qeT = singles.tile([D, G], BF16, tag="qeT")
    for src, dstT in (
        (io["K_gene"], kgT),
        (io["K_expr"], keT),
        (io["Q_gene"], qgT),
        (io["Q_expr"], qeT),
    ):
        big = ld.tile([P, NT, D], F32, tag="ld_t")
        nc.sync.dma_start(big[:], src.rearrange("(t p) d -> p t d", p=P))
        for j in range(4):  # 4 batches of 4 g-tiles
            ps = ps_t.tile([P, 4 * P], F32, tag="ps_t", name="ps_tr")[:D]
            for i in range(4):
                t = 4 * j + i
                nc.tensor.transpose(ps[:, i * P : (i + 1) * P], big[:, t, :], identity[:])
            nc.vector.tensor_copy(dstT[:, j * 512 : (j + 1) * 512], ps[:])

    # ---- fused projections -> K_fusedT / Q_fusedT [D, G] bf16 ----
    kfT = singles.tile([D, G], BF16, tag="kfT")
    qfT = singles.tile([D, G], BF16, tag="qfT")
    for gT, eT, wgT, weT, b_sb, fT in (
        (kgT, keT, wk_gTb, wk_eTb, wkb, kfT),
        (qgT, qeT, wq_gTb, wq_eTb, wqb, qfT),
    ):
        for n in range(4):
            ps = ps_o.tile([D, 512], F32, tag="ps_o", name="ps_pj")
            nc.tensor.matmul(
                ps[:], wgT[:], gT[:, n * 512 : (n + 1) * 512], start=True, stop=False
            )
            nc.tensor.matmul(
                ps[:], weT[:], eT[:, n * 512 : (n + 1) * 512], start=False, stop=True
            )
            nc.scalar.activation(
                fT[:, n * 512 : (n + 1) * 512], ps[:], AF.Identity, bias=b_sb[:, 0:1]
            )

    # ---- main attention loop (fully per-q-tile pipelined) ----
    mpool = ctx.enter_context(tc.tile_pool(name="mpool", bufs=3))
    epool = ctx.enter_context(tc.tile_pool(name="epool", bufs=2))
    empool = ctx.enter_context(tc.tile_pool(name="empool", bufs=2))
    tpool = ctx.enter_context(tc.tile_pool(name="tpool", bufs=2))
    opool = ctx.enter_context(tc.tile_pool(name="opool", bufs=2))
    rspool = ctx.enter_context(tc.tile_pool(name="rspool", bufs=4))

    m_ap = io["M"]
    out_r = io["out"].rearrange("(t p) d -> p t d", p=P)
    scale = 1.0 / np.sqrt(np.float32(D))

    # Per-qt state carried one step so the AV matmuls of qt-1 are emitted
    # between qt's logits and qt's transposes — PE chews on them while the
    # ScalarE/VectorE stages of qt run, instead of stalling at a group
    # barrier.
    pending = None  # (qt, emt, recip)

    def emit_av(pend):
        qt_p, emt_p, recip_p = pend
        # out[q, d] += expM^T_chunk.T @ V  (lhsT=emt chunk: 128 bf16 cols -> FWL)
        pso = ps_o.tile([P, D], F32, tag="ps_o", name="ps_av")
        for kt in range(NT):
            nc.tensor.matmul(
                pso[:],
                emt_p[:, kt, :],
                v_bf[:, kt, :],
                start=(kt == 0),
                stop=(kt == NT - 1),
            )
        ob = opool.tile([P, D], F32, tag="ob")
        # apply softmax denominator while copying out of PSUM
        nc.scalar.activation(
            ob[:], pso[:], AF.Copy, bias=0.0, scale=recip_p[:, 0:1]
        )
        nc.sync.dma_start(out_r[:, qt_p, :], ob[:])

    for qt in range(NT):
        mt = mpool.tile([P, G], F32, tag="m")
        nc.sync.dma_start(mt[:], m_ap[qt * P : (qt + 1) * P, :])

        ex = epool.tile([P, G], F32, tag="ex")
        rs = [rspool.tile([P, 1], F32, tag=f"rs{h}", name=f"rs{h}") for h in range(2)]
        # logits in two [128, 1024] psum tiles (2 banks each) so the next
        # q-tile's matmuls can start while this one's exp drains.
        for h in range(2):
            psl = ps_l.tile([P, 1024], F32, tag="ps_l")
            for n in range(2):
                nc.tensor.matmul(
                    psl[:, n * 512 : (n + 1) * 512],
                    qfT[:, qt * P : (qt + 1) * P],
                    kfT[:, (2 * h + n) * 512 : (2 * h + n + 1) * 512],
                    start=True,
                    stop=True,
                )
            nc.scalar.activation(
                ex[:, h * 1024 : (h + 1) * 1024],
                psl[:],
                AF.Exp,
                scale=float(scale),
                accum_out=rs[h][:],
            )
        rsum = rspool.tile([P, 1], F32, tag="rsum")
        nc.vector.tensor_add(rsum[:], rs[0][:], rs[1][:])
        recip = rspool.tile([P, 1], F32, tag="recip", name="recip")
        nc.vector.reciprocal(recip[:], rsum[:])

        em = empool.tile([P, G], BF16, tag="em")
        nc.vector.tensor_mul(em[:], ex[:], mt[:])

        # previous q-tile's AV runs on PE while this tile's exp/mult drain
        if pending is not None:
            emit_av(pending)

        emt = tpool.tile([P, NT, P], BF16, tag="emt")  # expM^T tiles [k, q]
        for j in range(4):
            pst = ps_t.tile([P, 4 * P], BF16, tag="ps_t")
            for k in range(4):
                kt = 4 * j + k
                nc.tensor.transpose(
                    pst[:, k * P : (k + 1) * P],
                    em[:, kt * P : (kt + 1) * P],
                    identity_bf[:],
                )
            # Pin the copy engine per k-group so each AV matmul's rhs
            # slice has a single writer engine (bounds its sync waits).
            if j < 2:
                nc.vector.tensor_copy(
                    emt[:, 4 * j : 4 * j + 4, :],
                    pst[:].rearrange("p (a b) -> p a b", a=4),
                )
            else:
                nc.scalar.copy(
                    emt[:, 4 * j : 4 * j + 4, :],
                    pst[:].rearrange("p (a b) -> p a b", a=4),
                )
        pending = (qt, emt, recip)

    emit_av(pending)


def _build():
    # Bacc (not plain Bass): its compile() legalizes sync waits
    # (move_matmul_waits_to_ldweights + generate_event_semaphores) which
    # walrus codegen requires (max 1 wait per instruction).
    nc = bacc.Bacc("TRN2", target_bir_lowering=False, debug=False)
    io = {}
    for name in ("Q_gene", "K_gene", "Q_expr", "K_expr", "V_expr"):
        io[name] = nc.dram_tensor(name, [G, D], F32, kind="ExternalInput").ap()
    io["M"] = nc.dram_tensor("M", [G, G], F32, kind="ExternalInput").ap()
    for name in ("WK_w", "WQ_w"):
        io[name] = nc.dram_tensor(name, [D, 2 * D], F32, kind="ExternalInput").ap()
    for name in ("WK_b", "WQ_b"):
        io[name] = nc.dram_tensor(name, [D], F32, kind="ExternalInput").ap()
    io["out"] = nc.dram_tensor("out", [G, D], F32, kind="ExternalOutput").ap()

    with tile.TileContext(nc) as tc:
        with ExitStack() as ctx:
            _emit(ctx, tc, io)
    nc.compile()
    return nc


_NC = None


def _get_nc():
    global _NC
    if _NC is None:
        _NC = _build()
    return _NC


def kernel(**inputs) -> np.ndarray:
    return run_kernel_with_results(**inputs)[0]


def run_kernel_with_results(trace=False, **inputs):
    """Returns (full_output, BassKernelResults)."""
    nc = _get_nc()
    per_core_names = ("Q_gene", "K_gene", "Q_expr", "K_expr", "V_expr", "M")
    shared_names = ("WK_w", "WK_b", "WQ_w", "WQ_b")
    arrs = {k: np.ascontiguousarray(np.asarray(v), dtype=np.float32) for k, v in inputs.items()}
    in_maps = []
    for c in range(N_CORES):
        im = {n: arrs[n][c] for n in per_core_names}
        for n in shared_names:
            im[n] = arrs[n]
        in_maps.append(im)
    res = run_bass_kernel_spmd(nc, in_maps, list(range(N_CORES)), trace=trace)
    out = np.stack([res.results[c]["out"] for c in range(N_CORES)], axis=0)
    return out.astype(np.float32), res


# revision 19
# speedup vs baseline: 2.6408x; 1.0807x over previous
"""Bass/Tile TRN2 kernel for nn_ExpressionAttentionLayer.

Math per batch b (B=8, G=2048, D=64):
    K_fused = concat([K_gene, K_expr], -1) @ WK_w.T + WK_b      # (G, D)
    Q_fused = concat([Q_gene, Q_expr], -1) @ WQ_w.T + WQ_b      # (G, D)
    A       = softmax(Q_fused @ K_fused.T / sqrt(D), axis=-1)
    out     = (A * M) @ V_expr                                   # (G, D)

Sharding: data-parallel over batch; core i handles batch i (B == n_cores == 8).
No collectives.

Per-core dataflow:
  - Transpose Q/K gene+expr into [D, G] via PE transpose-mode (1-pass for
    fp32 on cayman); project on PE in bf16 -> K_fusedT/Q_fusedT [64, G] bf16.
  - Per q-tile (128 rows): logits psum(fp32) = Q_tile.T @ K_fusedT (bf16,
    contract d=64), exp on ScalarE with fused row-sum (scale=1/sqrt(D)
    folded in; |logits| <~ 7 so no max-subtraction needed), multiply by the
    streamed M tile on VectorE (bf16 out), PE-transpose expM (bf16 psum),
    copy to [k, q] sbuf tiles, accumulate out^T over k-tiles on PE (bf16).
  - Apply the softmax denominator as a per-partition reciprocal scale while
    copying the re-transposed fp32 output, then DMA out.

fp32 matmuls cost 2 PE passes on trn2; every matmul here runs bf16 inputs
with fp32 PSUM accumulation except nothing — accuracy comes from fp32
softmax statistics and fp32 accumulation.
"""

from contextlib import ExitStack

import numpy as np

import concourse.bass as bass
import concourse.tile as tile
from concourse import bacc, mybir
from concourse.bass_utils import run_bass_kernel_spmd
from concourse.masks import make_identity

B, G, D = 8, 2048, 64
P = 128
NT = G // P  # 16 tiles of 128 rows
F32 = mybir.dt.float32
BF16 = mybir.dt.bfloat16
AF = mybir.ActivationFunctionType

N_CORES = 8


def _emit(ctx: ExitStack, tc: tile.TileContext, io: dict):
    nc = tc.nc

    singles = ctx.enter_context(tc.tile_pool(name="singles", bufs=1))
    ld = ctx.enter_context(tc.tile_pool(name="ld", bufs=4))

    # PSUM pools (8 banks total: 2*2 + 2 + 2 = 8)
    ps_l = ctx.enter_context(tc.tile_pool(name="ps_l", bufs=2, space="PSUM"))
    ps_t = ctx.enter_context(tc.tile_pool(name="ps_t", bufs=2, space="PSUM"))
    ps_o = ctx.enter_context(tc.tile_pool(name="ps_o", bufs=2, space="PSUM"))

    identity = singles.tile([P, P], F32)
    make_identity(nc, identity[:])
    identity_bf = singles.tile([P, P], BF16)
    nc.vector.tensor_copy(identity_bf[:], identity[:])

    # ---- HAM warmup: ~4us of junk matmuls while the first DMAs land, so
    # the PE clock is at 2.4 GHz when real work starts.
    junk = singles.tile([P, 512], BF16, tag="junk")
    nc.gpsimd.memset(junk[:], 0.0)
    for _ in range(10):
        psw = ps_l.tile([P, 1024], F32, tag="ps_l", name="ps_warm")
        nc.tensor.matmul(psw[:, 0:512], identity_bf[:], junk[:], start=True, stop=True)

    # ---- weights: WK_w/WQ_w are [D, 2D]; natural load, then PE-transpose the
    # two [64, 64] halves (base partition 0) and cast to bf16 lhsT tiles.
    wk_nat = singles.tile([D, 2 * D], F32, tag="wk_nat")
    wq_nat = singles.tile([D, 2 * D], F32, tag="wq_nat")
    nc.sync.dma_start(wk_nat[:], io["WK_w"][:, :])
    nc.sync.dma_start(wq_nat[:], io["WQ_w"][:, :])
    wk_gTb = singles.tile([D, D], BF16, tag="wk_gTb")
    wk_eTb = singles.tile([D, D], BF16, tag="wk_eTb")
    wq_gTb = singles.tile([D, D], BF16, tag="wq_gTb")
    wq_eTb = singles.tile([D, D], BF16, tag="wq_eTb")
    for nat, dsts in ((wk_nat, (wk_gTb, wk_eTb)), (wq_nat, (wq_gTb, wq_eTb))):
        for h, dst in enumerate(dsts):
            psw = ps_o.tile([P, D], F32, tag="ps_o", name="ps_w")
            nc.tensor.transpose(
                psw[:D, :], nat[:, h * D : (h + 1) * D], identity[:D, :D]
            )
            nc.vector.tensor_copy(dst[:], psw[:D, :])
    wkb = singles.tile([D, 1], F32, tag="wkb")
    wqb = singles.tile([D, 1], F32, tag="wqb")
    nc.sync.dma_start(wkb[:], io["WK_b"][:, None])
    nc.sync.dma_start(wqb[:], io["WQ_b"][:, None])

    # ---- V in [128, NT, D] (g on partitions), cast to bf16 for the AV matmul
    v_sb = singles.tile([P, NT, D], F32, tag="v")
    nc.sync.dma_start(v_sb[:], io["V_expr"].rearrange("(t p) d -> p t d", p=P))
    v_bf = singles.tile([P, NT, D], BF16, tag="v_bf")
    nc.vector.tensor_copy(v_bf[:], v_sb[:])

    # ---- transpose Q/K gene+expr into bf16 [D, G] (d on partitions) ----
    kgT = singles.tile([D, G], BF16, tag="kgT")
    keT = singles.tile([D, G], BF16, tag="keT")
    qgT = singles.tile([D, G], BF16, tag="qgT")
    qeT = singles.tile([D, G], BF16, tag="qeT")
    for src, dstT in (
        (io["K_gene"], kgT),
        (io["K_expr"], keT),
        (io["Q_gene"], qgT),
        (io["Q_expr"], qeT),
    ):
        big = ld.tile([P, NT, D], F32, tag="ld_t")
        nc.sync.dma_start(big[:], src.rearrange("(t p) d -> p t d", p=P))
        for j in range(4):  # 4 batches of 4 g-tiles
            ps = ps_t.tile([P, 4 * P], F32, tag="ps_t", name="ps_tr")[:D]
            for i in range(4):
                t = 4 * j + i
                nc.tensor.transpose(ps[:, i * P : (i + 1) * P], big[:, t, :], identity[:])
            if j % 2 == 0:
                nc.vector.tensor_copy(dstT[:, j * 512 : (j + 1) * 512], ps[:])
            else:
                nc.scalar.copy(dstT[:, j * 512 : (j + 1) * 512], ps[:])

    # ---- fused projections -> K_fusedT / Q_fusedT [D, G] bf16 ----
    kfT = singles.tile([D, G], BF16, tag="kfT")
    qfT = singles.tile([D, G], BF16, tag="qfT")
    for gT, eT, wgT, weT, b_sb, fT in (
        (kgT, keT, wk_gTb, wk_eTb, wkb, kfT),
        (qgT, qeT, wq_gTb, wq_eTb, wqb, qfT),
    ):
        for n in range(4):
            ps = ps_o.tile([D, 512], F32, tag="ps_o", name="ps_pj")
            nc.tensor.matmul(
                ps[:], wgT[:], gT[:, n * 512 : (n + 1) * 512], start=True, stop=False
            )
            nc.tensor.matmul(
                ps[:], weT[:], eT[:, n * 512 : (n + 1) * 512], start=False, stop=True
            )
            nc.scalar.activation(
                fT[:, n * 512 : (n + 1) * 512], ps[:], AF.Identity, bias=b_sb[:, 0:1]
            )

    # ---- main attention loop (fully per-q-tile pipelined) ----
    mpool = ctx.enter_context(tc.tile_pool(name="mpool", bufs=3))
    epool = ctx.enter_context(tc.tile_pool(name="epool", bufs=2))
    empool = ctx.enter_context(tc.tile_pool(name="empool", bufs=2))
    tpool = ctx.enter_context(tc.tile_pool(name="tpool", bufs=2))
    opool = ctx.enter_context(tc.tile_pool(name="opool", bufs=2))
    rspool = ctx.enter_context(tc.tile_pool(name="rspool", bufs=4))

    m_ap = io["M"]
    out_r = io["out"].rearrange("(t p) d -> p t d", p=P)
    scale = 1.0 / np.sqrt(np.float32(D))

    # Per-qt state carried one step so the AV matmuls of qt-1 are emitted
    # between qt's logits and qt's transposes — PE chews on them while the
    # ScalarE/VectorE stages of qt run, instead of stalling at a group
    # barrier.
    pending = None  # (qt, emt, recip)

    def emit_av(pend):
        qt_p, emt_p, recip_p = pend
        # out[q, d] += expM^T_chunk.T @ V  (lhsT=emt chunk: 128 bf16 cols -> FWL)
        pso = ps_o.tile([P, D], F32, tag="ps_o", name="ps_av")
        for kt in range(NT):
            nc.tensor.matmul(
                pso[:],
                emt_p[:, kt, :],
                v_bf[:, kt, :],
                start=(kt == 0),
                stop=(kt == NT - 1),
            )
        ob = opool.tile([P, D], F32, tag="ob")
        # apply softmax denominator while copying out of PSUM
        nc.scalar.activation(
            ob[:], pso[:], AF.Copy, bias=0.0, scale=recip_p[:, 0:1]
        )
        nc.sync.dma_start(out_r[:, qt_p, :], ob[:])

    for qt in range(NT):
        mt = mpool.tile([P, G], F32, tag="m")
        nc.sync.dma_start(mt[:], m_ap[qt * P : (qt + 1) * P, :])

        ex = epool.tile([P, G], F32, tag="ex")
        rs = [rspool.tile([P, 1], F32, tag=f"rs{h}", name=f"rs{h}") for h in range(2)]
        # logits in two [128, 1024] psum tiles (2 banks each) so the next
        # q-tile's matmuls can start while this one's exp drains.
        for h in range(2):
            psl = ps_l.tile([P, 1024], F32, tag="ps_l")
            for n in range(2):
                nc.tensor.matmul(
                    psl[:, n * 512 : (n + 1) * 512],
                    qfT[:, qt * P : (qt + 1) * P],
                    kfT[:, (2 * h + n) * 512 : (2 * h + n + 1) * 512],
                    start=True,
                    stop=True,
                )
            nc.scalar.activation(
                ex[:, h * 1024 : (h + 1) * 1024],
                psl[:],
                AF.Exp,
                scale=float(scale),
                accum_out=rs[h][:],
            )
        rsum = rspool.tile([P, 1], F32, tag="rsum")
        nc.vector.tensor_add(rsum[:], rs[0][:], rs[1][:])
        recip = rspool.tile([P, 1], F32, tag="recip", name="recip")
        nc.vector.reciprocal(recip[:], rsum[:])

        em = empool.tile([P, G], BF16, tag="em")
        nc.vector.tensor_mul(em[:], ex[:], mt[:])

        # previous q-tile's AV runs on PE while this tile's exp/mult drain
        if pending is not None:
            emit_av(pending)

        emt = tpool.tile([P, NT, P], BF16, tag="emt")  # expM^T tiles [k, q]
        for j in range(4):
            pst = ps_t.tile([P, 4 * P], BF16, tag="ps_t")
            for k in range(4):
                kt = 4 * j + k
                nc.tensor.transpose(
                    pst[:, k * P : (k + 1) * P],
                    em[:, kt * P : (kt + 1) * P],
                    identity_bf[:],
                )
            # Pin the copy engine per k-group so each AV matmul's rhs
            # slice has a single writer engine (bounds its sync waits).
            if j < 2:
                nc.vector.tensor_copy(
                    emt[:, 4 * j : 4 * j + 4, :],
                    pst[:].rearrange("p (a b) -> p a b", a=4),
                )
            else:
                nc.scalar.copy(
                    emt[:, 4 * j : 4 * j + 4, :],
                    pst[:].rearrange("p (a b) -> p a b", a=4),
                )
        pending = (qt, emt, recip)

    emit_av(pending)


def _build():
    # Bacc (not plain Bass): its compile() legalizes sync waits
    # (move_matmul_waits_to_ldweights + generate_event_semaphores) which
    # walrus codegen requires (max 1 wait per instruction).
    nc = bacc.Bacc("TRN2", target_bir_lowering=False, debug=False)
    io = {}
    for name in ("Q_gene", "K_gene", "Q_expr", "K_expr", "V_expr"):
        io[name] = nc.dram_tensor(name, [G, D], F32, kind="ExternalInput").ap()
    io["M"] = nc.dram_tensor("M", [G, G], F32, kind="ExternalInput").ap()
    for name in ("WK_w", "WQ_w"):
        io[name] = nc.dram_tensor(name, [D, 2 * D], F32, kind="ExternalInput").ap()
    for name in ("WK_b", "WQ_b"):
        io[name] = nc.dram_tensor(name, [D], F32, kind="ExternalInput").ap()
    io["out"] = nc.dram_tensor("out", [G, D], F32, kind="ExternalOutput").ap()

    with tile.TileContext(nc) as tc:
        with ExitStack() as ctx:
            _emit(ctx, tc, io)
    nc.compile()
    return nc


_NC = None


def _get_nc():
    global _NC
    if _NC is None:
        _NC = _build()
    return _NC


def kernel(**inputs) -> np.ndarray:
    return run_kernel_with_results(**inputs)[0]


def run_kernel_with_results(trace=False, **inputs):
    """Returns (full_output, BassKernelResults)."""
    nc = _get_nc()
    per_core_names = ("Q_gene", "K_gene", "Q_expr", "K_expr", "V_expr", "M")
    shared_names = ("WK_w", "WK_b", "WQ_w", "WQ_b")
    arrs = {k: np.ascontiguousarray(np.asarray(v), dtype=np.float32) for k, v in inputs.items()}
    in_maps = []
    for c in range(N_CORES):
        im = {n: arrs[n][c] for n in per_core_names}
        for n in shared_names:
            im[n] = arrs[n]
        in_maps.append(im)
    res = run_bass_kernel_spmd(nc, in_maps, list(range(N_CORES)), trace=trace)
    out = np.stack([res.results[c]["out"] for c in range(N_CORES)], axis=0)
    return out.astype(np.float32), res
